# revision 2
# baseline (speedup 1.0000x reference)
"""Trainium2 Bass kernel: multi-head causal attention with RoPE (LLaMA-style).

Problem: y = Attention(x) with B=2, S=2048, D=2048, H=16 heads, HD=128,
torch-Linear convention (y = x @ W.T), interleaved-rope, additive mask.

Sharding (8 NeuronCores): batch (2) x head-groups (4) grid.  Core c handles
batch b = c // 4 and heads 4g..4g+3 where g = c % 4 (tensor parallel:
wq/wk/wv column-parallel, wo row-parallel).  Each core returns a partial
y contribution [S, D]; the host sums the 4 partials per batch.

Layout strategy (no on-chip transposes anywhere):
  - Host pre-transposes: xT [D,S], wqT/wkT/wvT [D,E], woT [E,D].
  - Q^T,K^T computed directly in [hd, s] layout (hd = partitions) with the
    head-dim DEINTERLEAVED (rows 0-63 = even/"re" dims, 64-127 = odd/"im")
    by permuting wq/wk columns on the host; RoPE is then plain 64-partition
    elementwise ops.  The permutation is invisible to Q.K^T contraction.
  - scores are computed TRANSPOSED [sk, sq] so softmax-denominators come
    from a ones-matmul (column sums) and exp(scores)^T feeds the PV matmul
    directly as the moving operand: P^T never materializes.
  - attention out falls out as out^T [hd, sq] = exactly the stationary
    layout the wo row-parallel matmul wants.
Matmul inputs are bf16 (fp32 PSUM accumulation); softmax runs in fp32.
"""

import math
from contextlib import ExitStack

import numpy as np
import ml_dtypes

P = 128          # partitions / head dim
CW = 512         # s-chunk width (one PSUM bank of fp32)

_built_cache = {}


def _build(*, S, D, E, mask_mode):
    """Build + compile the SPMD Bass program for one core's shard.

    S: sequence length, D: model dim, E: head-columns per core (nH*128).
    mask_mode: 'causal' (use diag block + skip upper triangle),
               'none' (no mask, full attention),
               'generic' (arbitrary additive mask, applied everywhere).
    """
    import concourse.bacc as bacc
    import concourse.mybir as mybir
    import concourse.tile as tile

    f32 = mybir.dt.float32
    bf16 = mybir.dt.bfloat16
    Exp = mybir.ActivationFunctionType.Exp

    nDK = D // P       # k-tiles over model dim
    nH = E // P        # heads on this core
    nSC = S // CW      # 512-wide s-chunks
    nST = S // P       # 128-wide s-tiles
    TPC = CW // P      # s-tiles per chunk (4)
    SCALE = 1.0 / math.sqrt(P)
    causal = mask_mode == "causal"

    nc = bacc.Bacc("TRN2", target_bir_lowering=False, debug=False)

    xT = nc.dram_tensor("xT", [D, S], bf16, kind="ExternalInput").ap()
    wqT = nc.dram_tensor("wqT", [D, E], bf16, kind="ExternalInput").ap()
    wkT = nc.dram_tensor("wkT", [D, E], bf16, kind="ExternalInput").ap()
    wvT = nc.dram_tensor("wvT", [D, E], bf16, kind="ExternalInput").ap()
    woT = nc.dram_tensor("woT", [E, D], bf16, kind="ExternalInput").ap()
    cs = nc.dram_tensor("cs", [P, S], f32, kind="ExternalInput").ap()
    maskd = nc.dram_tensor("maskd", [P, P], f32, kind="ExternalInput").ap()
    if mask_mode == "generic":
        maskT = nc.dram_tensor("maskT", [S, S], bf16, kind="ExternalInput").ap()
    y = nc.dram_tensor("y", [S, D], f32, kind="ExternalOutput").ap()

    with tile.TileContext(nc) as tc, ExitStack() as ctx:
        const = ctx.enter_context(tc.tile_pool(name="const", bufs=1))
        tp = ctx.enter_context(tc.tile_pool(name="tmp", bufs=2))
        expp = ctx.enter_context(tc.tile_pool(name="expp", bufs=4))
        sbB = ctx.enter_context(tc.tile_pool(name="sbB", bufs=2))
        yp = ctx.enter_context(tc.tile_pool(name="yp", bufs=4))
        psA = ctx.enter_context(tc.tile_pool(name="psA", bufs=4, space="PSUM"))
        psB = ctx.enter_context(tc.tile_pool(name="psB", bufs=3, space="PSUM"))
        psD = ctx.enter_context(tc.tile_pool(name="psD", bufs=1, space="PSUM"))

        # ---- persistent tiles --------------------------------------------
        qt = const.tile([P, nH, S], bf16)    # rotated Q^T  (re rows 0-63)
        kt = const.tile([P, nH, S], bf16)    # rotated K^T
        v = const.tile([P, nST, E], bf16)    # V [s within tile, stile, e]
        outT = const.tile([P, nH, S], bf16)  # attention out^T per head
        cs_t = const.tile([P, S], f32)       # rows 0-63 cos^T, 64-127 sin^T
        md = const.tile([P, P], f32)         # diag mask block^T / SCALE
        ones_col = const.tile([P, 1], bf16)
        ones_row = const.tile([1, P], bf16)

        nc.vector.memset(ones_col, 1.0)
        nc.vector.memset(ones_row, 1.0)
        nc.sync.dma_start(out=cs_t, in_=cs)
        nc.sync.dma_start(out=md, in_=maskd)

        def rope(ps, dst, col):
            """ps: [128, CW] psum raw projection (re rows 0-63, im 64-127).
            dst: [128, CW] bf16 sbuf destination slice. col: s-slice."""
            re, im = ps[0:64, :], ps[64:128, :]
            cosv, sinv = cs_t[0:64, col], cs_t[64:128, col]
            t1 = tp.tile([64, CW], f32, tag="t1", name="t1")
            t2 = tp.tile([64, CW], f32, tag="t2", name="t2")
            nc.vector.tensor_mul(t1, re, cosv)
            nc.vector.tensor_mul(t2, im, sinv)
            nc.vector.tensor_sub(dst[0:64, :], t1, t2)
            t3 = tp.tile([64, CW], f32, tag="t1", name="t3")
            t4 = tp.tile([64, CW], f32, tag="t2", name="t4")
            nc.vector.tensor_mul(t3, re, sinv)
            nc.vector.tensor_mul(t4, im, cosv)
            nc.vector.tensor_add(dst[64:128, :], t3, t4)

        # ---- phase 1: Q^T / K^T / V projections --------------------------
        with tc.tile_pool(name="xw", bufs=1) as xtp, \
             tc.tile_pool(name="wz", bufs=3) as wpool:
            xt = xtp.tile([P, nDK, S], bf16)
            nc.sync.dma_start(out=xt, in_=xT.rearrange("(dk p) s -> p dk s", p=P))

            nKH = nDK // 2  # stream weights in two dk-halves
            for proj, (wdram, dest) in enumerate(((wqT, qt), (wkT, kt), (wvT, v))):
                wts = []
                for kh in range(2):
                    wt = wpool.tile([P, nKH, E], bf16, tag="w", name="wt")
                    nc.sync.dma_start(
                        out=wt,
                        in_=wdram.rearrange("(dk p) e -> p dk e", p=P)[
                            :, kh * nKH:(kh + 1) * nKH, :],
                    )
                    wts.append(wt)

                def wslice(dk, esl):
                    return wts[dk // nKH][:, dk % nKH, esl]

                if proj < 2:
                    for h in range(nH):
                        esl = slice(h * P, (h + 1) * P)
                        for sc in range(nSC):
                            col = slice(sc * CW, (sc + 1) * CW)
                            ps = psA.tile([P, CW], f32, tag="psA", name="ps_qk")
                            for dk in range(nDK):
                                nc.tensor.matmul(
                                    ps, wslice(dk, esl), xt[:, dk, col],
                                    start=(dk == 0), stop=(dk == nDK - 1))
                            rope(ps, dest[:, h, col], col)
                else:
                    for st in range(nST):
                        ps = psA.tile([P, CW], f32, tag="psA", name="ps_v")
                        for dk in range(nDK):
                            nc.tensor.matmul(
                                ps[:, 0:E], xt[:, dk, st * P:(st + 1) * P],
                                wslice(dk, slice(0, E)),
                                start=(dk == 0), stop=(dk == nDK - 1))
                        nc.scalar.copy(v[:, st, :], ps[:, 0:E])

        # ---- late pool (reuses xt/w space) -------------------------------
        late = ctx.enter_context(tc.tile_pool(name="late", bufs=1))
        wo_t = late.tile([P, nH, D], bf16)
        nc.sync.dma_start(out=wo_t, in_=woT.rearrange("(h p) d -> p h d", p=P))

        # ---- phase 2: attention ------------------------------------------
        for c in range(nSC):
            qcol = slice(c * CW, (c + 1) * CW)
            if mask_mode == "generic":
                mk = late.tile([P, nST, CW], bf16, tag="mk", name="mk", bufs=2)
                nc.sync.dma_start(
                    out=mk,
                    in_=maskT.rearrange("(j p) q -> p j q", p=P)[:, :, qcol])
            for h in range(nH):
                jmax = TPC * c + TPC - 1 if causal else nST - 1
                ps_o = psB.tile([P, CW], f32, tag="psB", name="ps_o")
                ps_d = psD.tile([1, CW], f32, tag="psD", name="ps_d")
                for j in range(jmax + 1):
                    o = max(0, j - TPC * c) * P if causal else 0
                    ps_s = psA.tile([P, CW], f32, tag="psA", name="ps_s")
                    nc.tensor.matmul(
                        ps_s[:, o:], kt[:, h, j * P:(j + 1) * P],
                        qt[:, h, c * CW + o:(c + 1) * CW],
                        start=True, stop=True)
                    if causal:
                        if j >= TPC * c:
                            nc.vector.tensor_add(
                                ps_s[:, o:o + P], ps_s[:, o:o + P], md)
                    elif mask_mode == "generic":
                        nc.vector.tensor_add(ps_s, ps_s, mk[:, j, :])
                    es = expp.tile([P, CW], bf16, tag="es", name="es")
                    nc.scalar.activation(es[:, o:], ps_s[:, o:], Exp, scale=SCALE)
                    nc.tensor.matmul(ps_d[:, o:], ones_col, es[:, o:],
                                     start=(j == 0), stop=(j == jmax))
                    nc.tensor.matmul(ps_o[:, o:], v[:, j, h * P:(h + 1) * P],
                                     es[:, o:], start=(j == 0), stop=(j == jmax))
                # normalize: out^T[:, sq] *= 1/denom[sq]
                rr = tp.tile([1, CW], f32, tag="rr", name="rr")
                nc.vector.reciprocal(rr, ps_d)
                bc = sbB.tile([P, CW], f32, tag="bc", name="bc")
                nc.gpsimd.partition_broadcast(out_ap=bc, in_ap=rr)
                nc.vector.tensor_mul(outT[:, h, qcol], ps_o, bc)

        # ---- phase 3: output projection (row-parallel partial) -----------
        nDC = D // CW
        for m in range(nST):
            for dc in range(nDC):
                ps_y = psA.tile([P, CW], f32, tag="psA", name="ps_y")
                for h in range(nH):
                    nc.tensor.matmul(
                        ps_y, outT[:, h, m * P:(m + 1) * P],
                        wo_t[:, h, dc * CW:(dc + 1) * CW],
                        start=(h == 0), stop=(h == nH - 1))
                yo = yp.tile([P, CW], f32, tag="yo", name="yo")
                nc.scalar.copy(yo, ps_y)
                nc.sync.dma_start(
                    out=y[m * P:(m + 1) * P, dc * CW:(dc + 1) * CW], in_=yo)

    nc.compile()
    return nc


def _get_built(mask_mode, S, D, E):
    key = (mask_mode, S, D, E)
    if key not in _built_cache:
        _built_cache[key] = _build(S=S, D=D, E=E, mask_mode=mask_mode)
    return _built_cache[key]


def _classify_mask(mask):
    S = mask.shape[0]
    if not mask.any():
        return "none"
    causal = np.where(np.triu(np.ones((S, S), dtype=bool), k=1),
                      np.float32(-1e9), np.float32(0.0))
    if np.array_equal(mask, causal):
        return "causal"
    return "generic"


def make_in_maps(x, wq, wk, wv, wo, freqs_cos, freqs_sin, mask, n_cores=8):
    """Host-side sharding + layout prep. Returns (in_maps, mask_mode, meta)."""
    bf = ml_dtypes.bfloat16
    x = np.asarray(x, np.float32)
    B, S, D = x.shape
    groups = n_cores // B
    E = D // groups
    nH = E // P
    scale = 1.0 / math.sqrt(P)

    mask = np.asarray(mask, np.float32)
    mode = _classify_mask(mask)

    fc = np.asarray(freqs_cos, np.float32)
    fs = np.asarray(freqs_sin, np.float32)
    cs = np.concatenate(
        [np.ascontiguousarray(fc.T), np.ascontiguousarray(fs.T)], axis=0
    ).astype(np.float32)                      # [128, S]
    maskd = np.ascontiguousarray(mask[0:P, 0:P].T / scale).astype(np.float32)

    # per-head deinterleave: head-local columns [0,2,...,126,1,3,...,127]
    perm1 = np.concatenate([np.arange(0, P, 2), np.arange(1, P, 2)])
    permE = np.concatenate([h * P + perm1 for h in range(nH)])

    wqT_f = np.asarray(wq, np.float32).T      # [D, D]
    wkT_f = np.asarray(wk, np.float32).T
    wvT_f = np.asarray(wv, np.float32).T
    woT_f = np.asarray(wo, np.float32).T      # [E_total, D]

    if mode == "generic":
        maskT_bf = np.ascontiguousarray(mask.T / scale).astype(bf)

    xT_b = [np.ascontiguousarray(x[b].T).astype(bf) for b in range(B)]

    in_maps = []
    for c in range(n_cores):
        b, g = divmod(c, groups)
        es = slice(g * E, (g + 1) * E)
        m = {
            "xT": xT_b[b],
            "wqT": np.ascontiguousarray(wqT_f[:, es][:, permE]).astype(bf),
            "wkT": np.ascontiguousarray(wkT_f[:, es][:, permE]).astype(bf),
            "wvT": np.ascontiguousarray(wvT_f[:, es]).astype(bf),
            "woT": np.ascontiguousarray(woT_f[es, :]).astype(bf),
            "cs": cs,
            "maskd": maskd,
        }
        if mode == "generic":
            m["maskT"] = maskT_bf
        in_maps.append(m)
    return in_maps, mode, (B, S, D, E, groups)


def kernel(x, wq, wk, wv, wo, freqs_cos, freqs_sin, mask, start_pos=0, **_):
    from concourse.bass_utils import run_bass_kernel_spmd

    in_maps, mode, (B, S, D, E, groups) = make_in_maps(
        x, wq, wk, wv, wo, freqs_cos, freqs_sin, mask)
    nc = _get_built(mode, S, D, E)
    res = run_bass_kernel_spmd(nc, in_maps, core_ids=list(range(len(in_maps))))
    parts = [r["y"] for r in res.results]
    out = np.stack(
        [np.sum(parts[b * groups:(b + 1) * groups], axis=0) for b in range(B)]
    ).astype(np.float32)
    return out


# revision 6
# speedup vs baseline: 1.0755x; 1.0755x over previous
"""Trainium2 Bass kernel: multi-head causal attention with RoPE (LLaMA-style).

Problem: y = Attention(x) with B=2, S=2048, D=2048, H=16 heads, HD=128,
torch-Linear convention (y = x @ W.T), interleaved-rope, additive mask.

Sharding (8 NeuronCores): batch (2) x head-groups (4) grid.  Core c handles
batch b = c // 4 and heads 4g..4g+3 where g = c % 4 (tensor parallel:
wq/wk/wv column-parallel, wo row-parallel).  Each core returns a partial
y contribution [S, D]; the host sums the 4 partials per batch.

Layout strategy (no on-chip transposes anywhere):
  - Host pre-transposes: xT [D,S], wqT/wkT/wvT [D,E], woT [E,D].
  - Q^T,K^T computed directly in [hd, s] layout (hd = partitions) with the
    head-dim DEINTERLEAVED (rows 0-63 = even/"re" dims, 64-127 = odd/"im")
    by permuting wq/wk columns on the host; RoPE is then plain 64-partition
    elementwise ops.  The permutation is invisible to Q.K^T contraction.
  - scores are computed TRANSPOSED [sk, sq] so softmax-denominators come
    from a ones-matmul (column sums) and exp(scores)^T feeds the PV matmul
    directly as the moving operand: P^T never materializes.
  - attention out falls out as out^T [hd, sq] = exactly the stationary
    layout the wo row-parallel matmul wants.
Matmul inputs are bf16 (fp32 PSUM accumulation); softmax runs in fp32.
"""

import math
from contextlib import ExitStack

import numpy as np
import ml_dtypes

P = 128          # partitions / head dim
CW = 512         # s-chunk width (one PSUM bank of fp32)

_built_cache = {}


def _build(*, S, D, E, mask_mode):
    """Build + compile the SPMD Bass program for one core's shard.

    S: sequence length, D: model dim, E: head-columns per core (nH*128).
    mask_mode: 'causal' (use diag block + skip upper triangle),
               'none' (no mask, full attention),
               'generic' (arbitrary additive mask, applied everywhere).
    """
    import concourse.bacc as bacc
    import concourse.mybir as mybir
    import concourse.tile as tile

    f32 = mybir.dt.float32
    bf16 = mybir.dt.bfloat16
    Exp = mybir.ActivationFunctionType.Exp

    nDK = D // P       # k-tiles over model dim
    nH = E // P        # heads on this core
    nSC = S // CW      # 512-wide s-chunks
    nST = S // P       # 128-wide s-tiles
    TPC = CW // P      # s-tiles per chunk (4)
    SCALE = 1.0 / math.sqrt(P)
    causal = mask_mode == "causal"

    nc = bacc.Bacc("TRN2", target_bir_lowering=False, debug=False)

    xT = nc.dram_tensor("xT", [D, S], bf16, kind="ExternalInput").ap()
    wqT = nc.dram_tensor("wqT", [D, E], bf16, kind="ExternalInput").ap()
    wkT = nc.dram_tensor("wkT", [D, E], bf16, kind="ExternalInput").ap()
    wvT = nc.dram_tensor("wvT", [D, E], bf16, kind="ExternalInput").ap()
    woT = nc.dram_tensor("woT", [E, D], bf16, kind="ExternalInput").ap()
    cs = nc.dram_tensor("cs", [P, S], f32, kind="ExternalInput").ap()
    maskd = nc.dram_tensor("maskd", [P, P], f32, kind="ExternalInput").ap()
    if mask_mode == "generic":
        maskT = nc.dram_tensor("maskT", [S, S], bf16, kind="ExternalInput").ap()
    y = nc.dram_tensor("y", [S, D], f32, kind="ExternalOutput").ap()

    with tile.TileContext(nc) as tc, ExitStack() as ctx:
        const = ctx.enter_context(tc.tile_pool(name="const", bufs=1))
        tp = ctx.enter_context(tc.tile_pool(name="tmp", bufs=2))
        expp = ctx.enter_context(tc.tile_pool(name="expp", bufs=6))
        sbB = ctx.enter_context(tc.tile_pool(name="sbB", bufs=2))
        yp = ctx.enter_context(tc.tile_pool(name="yp", bufs=4))
        psA = ctx.enter_context(tc.tile_pool(name="psA", bufs=4, space="PSUM"))
        psB = ctx.enter_context(tc.tile_pool(name="psB", bufs=3, space="PSUM"))
        psD = ctx.enter_context(tc.tile_pool(name="psD", bufs=1, space="PSUM"))

        # ---- persistent tiles --------------------------------------------
        qt = const.tile([P, nH, S], bf16)    # rotated Q^T  (re rows 0-63)
        kt = const.tile([P, nH, S], bf16)    # rotated K^T
        v = const.tile([P, nST, E], bf16)    # V [s within tile, stile, e]
        outT = const.tile([P, nH, S], bf16)  # attention out^T per head
        cs_t = const.tile([P, S], f32)       # rows 0-63 cos^T, 64-127 sin^T
        md = const.tile([P, P], f32)         # diag mask block^T / SCALE
        ones_col = const.tile([P, 1], bf16)
        ones_row = const.tile([1, P], bf16)

        nc.vector.memset(ones_col, 1.0)
        nc.vector.memset(ones_row, 1.0)
        nc.sync.dma_start(out=cs_t, in_=cs)
        nc.sync.dma_start(out=md, in_=maskd)

        def rope(ps, dst, col):
            """ps: [128, CW] psum raw projection (re rows 0-63, im 64-127).
            dst: [128, CW] bf16 sbuf destination slice. col: s-slice."""
            re, im = ps[0:64, :], ps[64:128, :]
            cosv, sinv = cs_t[0:64, col], cs_t[64:128, col]
            t1 = tp.tile([64, CW], f32, tag="t1", name="t1")
            t2 = tp.tile([64, CW], f32, tag="t2", name="t2")
            nc.vector.tensor_mul(t1, re, cosv)
            nc.vector.tensor_mul(t2, im, sinv)
            nc.vector.tensor_sub(dst[0:64, :], t1, t2)
            t3 = tp.tile([64, CW], f32, tag="t1", name="t3")
            t4 = tp.tile([64, CW], f32, tag="t2", name="t4")
            nc.vector.tensor_mul(t3, re, sinv)
            nc.vector.tensor_mul(t4, im, cosv)
            nc.vector.tensor_add(dst[64:128, :], t3, t4)

        # ---- phase 1: Q^T / K^T / V projections --------------------------
        with tc.tile_pool(name="xw", bufs=1) as xtp, \
             tc.tile_pool(name="wz", bufs=3) as wpool:
            nKH = nDK // 2
            # x^T in two dk-half tiles so the first matmuls only wait on
            # half the 8.4MB transfer
            xts = []
            for kh in range(2):
                xt = xtp.tile([P, nKH, S], bf16, tag=f"xt{kh}", name="xt")
                nc.sync.dma_start(
                    out=xt,
                    in_=xT.rearrange("(dk p) s -> p dk s", p=P)[
                        :, kh * nKH:(kh + 1) * nKH, :])
                xts.append(xt)

            def xslice(dk, ssl):
                return xts[dk // nKH][:, dk % nKH, ssl]

            for proj, (wdram, dest) in enumerate(((wqT, qt), (wkT, kt), (wvT, v))):
                wts = []
                for kh in range(2):
                    wt = wpool.tile([P, nKH, E], bf16, tag="w", name="wt")
                    nc.sync.dma_start(
                        out=wt,
                        in_=wdram.rearrange("(dk p) e -> p dk e", p=P)[
                            :, kh * nKH:(kh + 1) * nKH, :],
                    )
                    wts.append(wt)

                def wslice(dk, esl):
                    return wts[dk // nKH][:, dk % nKH, esl]

                if proj < 2:
                    for h in range(nH):
                        esl = slice(h * P, (h + 1) * P)
                        for sc in range(nSC):
                            col = slice(sc * CW, (sc + 1) * CW)
                            ps = psA.tile([P, CW], f32, tag="psA", name="ps_qk")
                            for dk in range(nDK):
                                nc.tensor.matmul(
                                    ps, wslice(dk, esl), xslice(dk, col),
                                    start=(dk == 0), stop=(dk == nDK - 1))
                            rope(ps, dest[:, h, col], col)
                else:
                    for st in range(nST):
                        ssl = slice(st * P, (st + 1) * P)
                        ps = psA.tile([P, CW], f32, tag="psA", name="ps_v")
                        for dk in range(nDK):
                            nc.tensor.matmul(
                                ps[:, 0:E], xslice(dk, ssl),
                                wslice(dk, slice(0, E)),
                                start=(dk == 0), stop=(dk == nDK - 1))
                        nc.scalar.copy(v[:, st, :], ps[:, 0:E])

        # ---- late pool (reuses xt/w space) -------------------------------
        late = ctx.enter_context(tc.tile_pool(name="late", bufs=1))
        wo_t = late.tile([P, nH, D], bf16)
        nc.sync.dma_start(out=wo_t, in_=woT.rearrange("(h p) d -> p h d", p=P))

        # ---- phase 2: attention ------------------------------------------
        for c in range(nSC):
            qcol = slice(c * CW, (c + 1) * CW)
            if mask_mode == "generic":
                mk = late.tile([P, nST, CW], bf16, tag="mk", name="mk", bufs=2)
                nc.sync.dma_start(
                    out=mk,
                    in_=maskT.rearrange("(j p) q -> p j q", p=P)[:, :, qcol])
            for h in range(nH):
                jmax = TPC * c + TPC - 1 if causal else nST - 1
                ps_o = psB.tile([P, CW], f32, tag="psB", name="ps_o")
                ps_d = psD.tile([1, CW], f32, tag="psD", name="ps_d")
                for j in range(jmax + 1):
                    o = max(0, j - TPC * c) * P if causal else 0
                    ps_s = psA.tile([P, CW], f32, tag="psA", name="ps_s")
                    nc.tensor.matmul(
                        ps_s[:, o:], kt[:, h, j * P:(j + 1) * P],
                        qt[:, h, c * CW + o:(c + 1) * CW],
                        start=True, stop=True)
                    if causal:
                        if j >= TPC * c:
                            nc.vector.tensor_add(
                                ps_s[:, o:o + P], ps_s[:, o:o + P], md)
                    elif mask_mode == "generic":
                        nc.vector.tensor_add(ps_s, ps_s, mk[:, j, :])
                    es = expp.tile([P, CW], bf16, tag="es", name="es")
                    nc.scalar.activation(es[:, o:], ps_s[:, o:], Exp, scale=SCALE)
                    nc.tensor.matmul(ps_d[:, o:], ones_col, es[:, o:],
                                     start=(j == 0), stop=(j == jmax))
                    nc.tensor.matmul(ps_o[:, o:], v[:, j, h * P:(h + 1) * P],
                                     es[:, o:], start=(j == 0), stop=(j == jmax))
                # normalize: out^T[:, sq] *= 1/denom[sq].
                # Copy denom out fast (frees the psD bank), broadcast it,
                # and take the reciprocal on all 128 partitions (a [1,CW]
                # reciprocal runs on a single DVE lane: ~3.3us vs ~0.6us).
                dd = tp.tile([1, CW], f32, tag="rr", name="dd")
                nc.scalar.copy(dd, ps_d)
                bc = sbB.tile([P, CW], f32, tag="bc", name="bc")
                nc.gpsimd.partition_broadcast(out_ap=bc, in_ap=dd)
                bcr = sbB.tile([P, CW], f32, tag="bcr", name="bcr")
                nc.vector.reciprocal(bcr, bc)
                nc.vector.tensor_mul(outT[:, h, qcol], ps_o, bcr)

        # ---- phase 3: output projection (row-parallel partial) -----------
        nDC = D // CW
        for m in range(nST):
            for dc in range(nDC):
                ps_y = psA.tile([P, CW], f32, tag="psA", name="ps_y")
                for h in range(nH):
                    nc.tensor.matmul(
                        ps_y, outT[:, h, m * P:(m + 1) * P],
                        wo_t[:, h, dc * CW:(dc + 1) * CW],
                        start=(h == 0), stop=(h == nH - 1))
                yo = yp.tile([P, CW], f32, tag="yo", name="yo")
                nc.scalar.copy(yo, ps_y)
                nc.sync.dma_start(
                    out=y[m * P:(m + 1) * P, dc * CW:(dc + 1) * CW], in_=yo)

    nc.compile()
    return nc


def _get_built(mask_mode, S, D, E):
    key = (mask_mode, S, D, E)
    if key not in _built_cache:
        _built_cache[key] = _build(S=S, D=D, E=E, mask_mode=mask_mode)
    return _built_cache[key]


def _classify_mask(mask):
    S = mask.shape[0]
    if not mask.any():
        return "none"
    causal = np.where(np.triu(np.ones((S, S), dtype=bool), k=1),
                      np.float32(-1e9), np.float32(0.0))
    if np.array_equal(mask, causal):
        return "causal"
    return "generic"


def make_in_maps(x, wq, wk, wv, wo, freqs_cos, freqs_sin, mask, n_cores=8):
    """Host-side sharding + layout prep. Returns (in_maps, mask_mode, meta)."""
    bf = ml_dtypes.bfloat16
    x = np.asarray(x, np.float32)
    B, S, D = x.shape
    groups = n_cores // B
    E = D // groups
    nH = E // P
    scale = 1.0 / math.sqrt(P)

    mask = np.asarray(mask, np.float32)
    mode = _classify_mask(mask)

    fc = np.asarray(freqs_cos, np.float32)
    fs = np.asarray(freqs_sin, np.float32)
    cs = np.concatenate(
        [np.ascontiguousarray(fc.T), np.ascontiguousarray(fs.T)], axis=0
    ).astype(np.float32)                      # [128, S]
    maskd = np.ascontiguousarray(mask[0:P, 0:P].T / scale).astype(np.float32)

    # per-head deinterleave: head-local columns [0,2,...,126,1,3,...,127]
    perm1 = np.concatenate([np.arange(0, P, 2), np.arange(1, P, 2)])
    permE = np.concatenate([h * P + perm1 for h in range(nH)])

    wqT_f = np.asarray(wq, np.float32).T      # [D, D]
    wkT_f = np.asarray(wk, np.float32).T
    wvT_f = np.asarray(wv, np.float32).T
    woT_f = np.asarray(wo, np.float32).T      # [E_total, D]

    if mode == "generic":
        maskT_bf = np.ascontiguousarray(mask.T / scale).astype(bf)

    xT_b = [np.ascontiguousarray(x[b].T).astype(bf) for b in range(B)]

    in_maps = []
    for c in range(n_cores):
        b, g = divmod(c, groups)
        es = slice(g * E, (g + 1) * E)
        m = {
            "xT": xT_b[b],
            "wqT": np.ascontiguousarray(wqT_f[:, es][:, permE]).astype(bf),
            "wkT": np.ascontiguousarray(wkT_f[:, es][:, permE]).astype(bf),
            "wvT": np.ascontiguousarray(wvT_f[:, es]).astype(bf),
            "woT": np.ascontiguousarray(woT_f[es, :]).astype(bf),
            "cs": cs,
            "maskd": maskd,
        }
        if mode == "generic":
            m["maskT"] = maskT_bf
        in_maps.append(m)
    return in_maps, mode, (B, S, D, E, groups)


def kernel(x, wq, wk, wv, wo, freqs_cos, freqs_sin, mask, start_pos=0, **_):
    from concourse.bass_utils import run_bass_kernel_spmd

    in_maps, mode, (B, S, D, E, groups) = make_in_maps(
        x, wq, wk, wv, wo, freqs_cos, freqs_sin, mask)
    nc = _get_built(mode, S, D, E)
    res = run_bass_kernel_spmd(nc, in_maps, core_ids=list(range(len(in_maps))))
    parts = [r["y"] for r in res.results]
    out = np.stack(
        [np.sum(parts[b * groups:(b + 1) * groups], axis=0) for b in range(B)]
    ).astype(np.float32)
    return out
